# revision 1
# baseline (speedup 1.0000x reference)
"""CLSTMCell fused cell kernel for 8 Trainium2 NeuronCores.

Data-parallel over the batch: each of the 8 cores processes a 512-row batch
shard; the four (D,4U) kernels and biases are replicated to every core.

Math (per batch shard, D = U = 1024):
    zr = xr@R + xi@I + hr@Rr + hi@Ir + br          [512, 4096]
    zi = xi@R - xr@I + hi@Rr - hr@Ir + bi          [512, 4096]
    per gate g (i,f,c,o) and half (r from zr, i from zi):
        i,f,o -> hard_sigmoid(z) = clip(0.2 z + 0.5, 0, 1);  c~ -> tanh(z)
    c = f*c_tm1 + i*tanh(c~);  h = o*tanh(c)

Device layout: output columns (n) on SBUF partitions, batch on the free dim.
Each matmul takes a [128k, 128n] weight tile as the stationary operand and a
transposed-activation block [128k, 512b] as the moving operand at float32r
(full-rate fp32), accumulating zT[n0:n0+128, :] over the 32 k-blocks. The
zr/zi pair shares each loaded weight tile, amortizing LDWEIGHTS. One phase =
one 128-wide u-block: all four gate psums for both halves live in the 8 PSUM
banks, so the gate combine needs no cross-phase staging, and biases are
per-partition scalars. Host-side work is layout only (slice/transpose/
reshape); zi's -xr/-hr blocks are negated once on device.
"""

import sys

sys.path.insert(0, "/opt/trn_rl_repo")

import numpy as np

import concourse.bacc as bacc
import concourse.mybir as mybir
import concourse.tile as tile
from concourse.bass_utils import run_bass_kernel_spmd

N_CORES = 8
B, D, U = 4096, 1024, 1024
BS = B // N_CORES          # batch rows per core
P = 128                    # SBUF partitions
KB = (2 * D + 2 * U) // P  # 32 contraction blocks of 128
NJ = U // P                # 8 u-blocks (phases)
F32 = mybir.dt.float32
F32R = mybir.dt.float32r
ADD = mybir.AluOpType.add
MULT = mybir.AluOpType.mult
MIN = mybir.AluOpType.min
MAX = mybir.AluOpType.max
TANH = None  # set lazily (mybir.ActivationFunctionType.Tanh)

_CACHE = {}


def _build():
    nc = bacc.Bacc("TRN2", target_bir_lowering=False, debug=False,
                   num_devices=N_CORES)
    Tanh = mybir.ActivationFunctionType.Tanh

    din = {}
    for name in ("xrT", "xiT", "hrT", "hiT"):
        din[name] = nc.dram_tensor(name, [D, BS], F32R, kind="ExternalInput").ap()
    din["c_prevT"] = nc.dram_tensor("c_prevT", [2 * U, BS], F32,
                                    kind="ExternalInput").ap()
    din["wperm"] = nc.dram_tensor("wperm", [NJ * KB * P, 4 * P], F32R,
                                  kind="ExternalInput").ap()
    din["brT"] = nc.dram_tensor("brT", [P, KB], F32, kind="ExternalInput").ap()
    din["biT"] = nc.dram_tensor("biT", [P, KB], F32, kind="ExternalInput").ap()
    h_outT = nc.dram_tensor("h_outT", [2 * U, BS], F32, kind="ExternalOutput").ap()
    c_outT = nc.dram_tensor("c_outT", [2 * U, BS], F32, kind="ExternalOutput").ap()

    with tile.TileContext(nc) as tc:
        with (
            tc.tile_pool(name="acts", bufs=48) as acts,
            tc.tile_pool(name="bias", bufs=4) as bias_p,
            tc.tile_pool(name="wpool", bufs=20) as wpool,
            tc.tile_pool(name="cprev", bufs=6) as cpool,
            tc.tile_pool(name="gat", bufs=6) as gat_p,
            tc.tile_pool(name="tmp", bufs=6) as tmp_p,
            tc.tile_pool(name="outs", bufs=8) as out_p,
            tc.tile_pool(name="psum", bufs=8, space="PSUM") as psum_p,
        ):
            # --- resident transposed-activation blocks, loaded lazily -------
            act_tiles = {}   # (src_name, block) -> tile
            neg_tiles = {}

            def act(name, j):
                t = act_tiles.get((name, j))
                if t is None:
                    t = acts.tile([P, BS], F32R, tag="acts", name=f"{name}{j}")
                    nc.sync.dma_start(t[:], din[name][j * P:(j + 1) * P, :])
                    act_tiles[(name, j)] = t
                return t

            def nact(name, j):
                t = neg_tiles.get((name, j))
                if t is None:
                    t = acts.tile([P, BS], F32R, tag="acts", name=f"n{name}{j}")
                    nc.vector.tensor_scalar_mul(t[:], act(name, j)[:], -1.0)
                    neg_tiles[(name, j)] = t
                return t

            A_SRC = ("xrT", "xiT", "hrT", "hiT")   # zr moving blocks by k//8
            B_SRC = ("xiT", "xrT", "hiT", "hrT")   # zi moving blocks (neg on 1,3)

            def a_block(k):
                return act(A_SRC[k // 8], k % 8)

            def b_block(k):
                name = B_SRC[k // 8]
                if (k // 8) % 2 == 1:
                    return nact(name, k % 8)
                return act(name, k % 8)

            # --- per-partition bias tiles [128, 32]; col m = n-block index --
            # raw for the c~ gate; 0.2*b + 0.5 pre-folded for the hsig gates.
            # Emitted lazily (first combine) so startup DMAs aren't queued
            # behind them.
            braw, bhs = [], []

            def emit_bias():
                for name in ("brT", "biT"):
                    t = bias_p.tile([P, KB], F32, tag="bias",
                                    name=f"braw_{name}")
                    nc.sync.dma_start(t[:], din[name][:, :])
                    braw.append(t)
                    t2 = bias_p.tile([P, KB], F32, tag="bias",
                                     name=f"bhs_{name}")
                    nc.vector.tensor_scalar(t2[:], t[:], 0.2, 0.5, MULT, ADD)
                    bhs.append(t2)

            # prime the first few moving blocks so phase 0's thin opening
            # k-steps (3 matmuls each) don't run ahead of the DMA stream
            for kk in range(3):
                a_block(kk)
                b_block(kk)

            # --- main loop: one phase per 128-wide u-block ------------------
            for j in range(NJ):
                # psum groups: (gate, z) -> zT[g*U + j*128 : .. , :] (8 banks)
                ps = {(g, z): psum_p.tile([P, BS], F32, tag="ps",
                                          name=f"ps_{j}_{g}_{z}")
                      for g in range(4) for z in range(2)}
                # --- gate combine, per half (emitted via combine()) -------
                cps = {}

                def emit_cps(j=j):
                    for z in range(2):
                        rows0 = z * U + j * P
                        cp = cpool.tile([P, BS], F32, tag="cprev",
                                        name=f"cp_{j}_{z}")
                        nc.sync.dma_start(
                            cp[:], din["c_prevT"][rows0:rows0 + P, :])
                        cps[z] = cp

                tc2s = {}

                def combine_ci(z, j=j, ps=ps):
                    if not braw:
                        emit_bias()
                    rows0 = z * U + j * P
                    cp = cps[z]

                    def relugate(g):
                        # relu(0.2*z + (0.2*b + 0.5)) on ACT straight from
                        # PSUM; the min(.,1) rides the consuming DVE op
                        t = gat_p.tile([P, BS], F32, tag="gat",
                                       name=f"hs_{j}_{z}_{g}")
                        bia = bhs[z][:, g * NJ + j:g * NJ + j + 1]
                        nc.scalar.activation(
                            t[:], ps[(g, z)][:],
                            mybir.ActivationFunctionType.Relu,
                            bias=bia, scale=0.2)
                        return t

                    # c~ = tanh(z_c + b_c), bias applied inside the ACT op
                    tct = tmp_p.tile([P, BS], F32, tag="tmp",
                                     name=f"tct_{j}_{z}")
                    nc.scalar.activation(
                        tct[:], ps[(2, z)][:], Tanh,
                        bias=braw[z][:, 2 * NJ + j:2 * NJ + j + 1], scale=1.0)

                    f_t = relugate(1)
                    i_t = relugate(0)
                    # c = min(f,1)*c_prev + min(i,1)*tanh(c~)
                    t1 = tmp_p.tile([P, BS], F32, tag="tmp", name=f"t1_{j}_{z}")
                    nc.vector.scalar_tensor_tensor(
                        t1[:], f_t[:], 1.0, cp[:], MIN, MULT)
                    t2 = tmp_p.tile([P, BS], F32, tag="tmp", name=f"t2_{j}_{z}")
                    nc.vector.scalar_tensor_tensor(
                        t2[:], i_t[:], 1.0, tct[:], MIN, MULT)
                    cn = out_p.tile([P, BS], F32, tag="out", name=f"cn_{j}_{z}")
                    nc.vector.tensor_tensor(cn[:], t1[:], t2[:], ADD)
                    nc.sync.dma_start(c_outT[rows0:rows0 + P, :], cn[:])
                    tc2 = tmp_p.tile([P, BS], F32, tag="tmp", name=f"tc2_{j}_{z}")
                    nc.scalar.activation(tc2[:], cn[:], Tanh)
                    tc2s[z] = (tc2, relugate)

                def combine_o(z, j=j, ps=ps):
                    rows0 = z * U + j * P
                    tc2, relugate = tc2s[z]
                    if j == NJ - 1 and z == 1:
                        # kernel tail: half-batch chunks pipeline the ACT
                        # relu, DVE mul and h DMA instead of serializing
                        # three full-width ops after the last matmul
                        o_t = gat_p.tile([P, BS], F32, tag="gat",
                                         name=f"hsl_{j}_{z}")
                        hn = out_p.tile([P, BS], F32, tag="out",
                                        name=f"hn_{j}_{z}")
                        bia = bhs[z][:, 3 * NJ + j:3 * NJ + j + 1]
                        for h0 in (0, BS // 2):
                            sl = slice(h0, h0 + BS // 2)
                            nc.scalar.activation(
                                o_t[:, sl], ps[(3, z)][:, sl],
                                mybir.ActivationFunctionType.Relu,
                                bias=bia, scale=0.2)
                            nc.vector.scalar_tensor_tensor(
                                hn[:, sl], o_t[:, sl], 1.0, tc2[:, sl],
                                MIN, MULT)
                            nc.sync.dma_start(
                                h_outT[rows0:rows0 + P, sl], hn[:, sl])
                        return
                    o_t = relugate(3)
                    hn = out_p.tile([P, BS], F32, tag="out", name=f"hn_{j}_{z}")
                    nc.vector.scalar_tensor_tensor(
                        hn[:], o_t[:], 1.0, tc2[:], MIN, MULT)
                    nc.sync.dma_start(h_outT[rows0:rows0 + P, :], hn[:])

                # staggered k-sweeps per group class: f/c~/i run at lag
                # 0 (real) / 4 (imag); the o-gate groups trail at lag 8 / 12,
                # so after the very last matmul only the short o->h chain
                # remains, and each half's c-chain hides under later matmuls
                LAG, OLAG = 4, 8
                wts = {}
                for t in range(KB + OLAG + LAG):
                    if t < KB:
                        k = t
                        if j == 0:
                            a_block(k)  # first-use DMA just ahead of its step
                        wt = wpool.tile([P, 4 * P], F32R, tag="w",
                                        name=f"w_{j}_{k}")
                        row0 = (j * KB + k) * P
                        # first weight tiles issue from ACT's HWDGE so they
                        # aren't queued behind the priming act-DMAs on SP
                        weng = nc.scalar if (j == 0 and k < 2) else nc.sync
                        weng.dma_start(wt[:],
                                       din["wperm"][row0:row0 + P, :])
                        wts[k] = wt
                        am = a_block(k)[:]
                        for g in (0, 1, 2):
                            nc.tensor.matmul(ps[(g, 0)][:],
                                             wt[:, g * P:(g + 1) * P], am,
                                             start=(k == 0), stop=(k == KB - 1))
                    if LAG <= t < KB + LAG:
                        k = t - LAG
                        if j == 0:
                            b_block(k)
                        bm = b_block(k)[:]
                        wt = wts[k]
                        for g in (0, 1, 2):
                            nc.tensor.matmul(ps[(g, 1)][:],
                                             wt[:, g * P:(g + 1) * P], bm,
                                             start=(k == 0), stop=(k == KB - 1))
                    if OLAG <= t < KB + OLAG:
                        k = t - OLAG
                        wt = wts[k]
                        nc.tensor.matmul(ps[(3, 0)][:],
                                         wt[:, 3 * P:4 * P], a_block(k)[:],
                                         start=(k == 0), stop=(k == KB - 1))
                    if OLAG + LAG <= t < KB + OLAG + LAG:
                        k = t - OLAG - LAG
                        wt = wts.pop(k)
                        nc.tensor.matmul(ps[(3, 1)][:],
                                         wt[:, 3 * P:4 * P], b_block(k)[:],
                                         start=(k == 0), stop=(k == KB - 1))
                    if t == 2:
                        emit_cps()
                    if t == KB - 1:
                        combine_ci(0)
                    if t == KB + LAG - 1:
                        combine_ci(1)
                    if t == KB + OLAG - 1:
                        combine_o(0)
                combine_o(1)

    nc.compile()
    return nc


def _in_maps(inputs, h_tm1, c_tm1, wr, wi, wrr, wir, br, bi):
    brT = np.ascontiguousarray(br.reshape(KB, P).T)
    biT = np.ascontiguousarray(bi.reshape(KB, P).T)
    # wperm[j, k, p, g*128+c] = W_src(k)[(k%8)*128+p, g*1024+j*128+c]
    wall = np.stack([wr, wi, wrr, wir])        # [s, 1024, 4096]
    v = wall.reshape(4, 8, P, 4, NJ, P)        # [s, kr, p, g, j, c]
    wperm = np.ascontiguousarray(
        v.transpose(4, 0, 1, 2, 3, 5).reshape(NJ * KB * P, 4 * P))
    maps = []
    for c in range(N_CORES):
        rows = slice(c * BS, (c + 1) * BS)
        maps.append({
            "xrT": np.ascontiguousarray(inputs[rows, :D].T),
            "xiT": np.ascontiguousarray(inputs[rows, D:].T),
            "hrT": np.ascontiguousarray(h_tm1[rows, :U].T),
            "hiT": np.ascontiguousarray(h_tm1[rows, U:].T),
            "c_prevT": np.ascontiguousarray(c_tm1[rows].T),
            "wperm": wperm,
            "brT": brT, "biT": biT,
        })
    return maps


def kernel(inputs, h_tm1, c_tm1, real_kernel, imaginary_kernel,
           real_recurrent_kernel, imaginary_recurrent_kernel,
           real_bias, imaginary_bias):
    if "nc" not in _CACHE:
        _CACHE["nc"] = _build()
    nc = _CACHE["nc"]

    maps = _in_maps(
        np.ascontiguousarray(inputs, dtype=np.float32),
        np.ascontiguousarray(h_tm1, dtype=np.float32),
        np.ascontiguousarray(c_tm1, dtype=np.float32),
        np.ascontiguousarray(real_kernel, dtype=np.float32),
        np.ascontiguousarray(imaginary_kernel, dtype=np.float32),
        np.ascontiguousarray(real_recurrent_kernel, dtype=np.float32),
        np.ascontiguousarray(imaginary_recurrent_kernel, dtype=np.float32),
        np.ascontiguousarray(real_bias, dtype=np.float32),
        np.ascontiguousarray(imaginary_bias, dtype=np.float32),
    )
    res = run_bass_kernel_spmd(nc, maps, list(range(N_CORES)))
    h = np.concatenate(
        [res.results[c]["h_outT"].T for c in range(N_CORES)], axis=0)
    c = np.concatenate(
        [res.results[c]["c_outT"].T for c in range(N_CORES)], axis=0)
    return np.ascontiguousarray(h), np.ascontiguousarray(c)



# revision 4
# speedup vs baseline: 1.3057x; 1.3057x over previous
"""CLSTMCell fused cell kernel for 8 Trainium2 NeuronCores.

Data-parallel over the batch: each of the 8 cores processes a 512-row batch
shard; weights and biases are replicated to every core.

Complex-multiply structure (z = x·(R - iI) + h·(Rr - iIr) + b) is computed
with Gauss's 3-multiplication trick instead of 4 real matmuls:
    k1  = (xr+xi)@R + (hr+hi)@Rr
    k2  = xr@(-(I+R)) + hr@(-(Ir+Rr))
    k3n = xi@(I-R)    + hi@(Ir-Rr)
    zr = k1 + k3n + br          [512, 4096]
    zi = k1 + k2  + bi          [512, 4096]
This cuts tensor-engine work by 25%. All matmul operands are fp16 (e5m10),
which halves weight DMA vs fp32 while keeping the end-to-end max-rel error
at ~2e-3 (measured on the reference data), well inside the 2e-2 gate. The
weight combinations and the (x+xi)/(h+hi) sums are precomputed on host.

Device layout: output columns (n) on SBUF partitions, batch on the free dim.
One phase = one (128-wide u-block, gate-pair): pair0=(f,c~), pair1=(i,o).
Per gate, the three Gauss accumulation chains live in 3 PSUM banks (6 banks
per pair-phase); the chains contract over 16 k-blocks of 128. The gate
combine reads two PSUM banks per z-half with one DVE add, then applies the
activation on ACT with fused scale+bias. Gate gl=1 trails gl=0 by GLAG
k-steps so combines overlap the next chains' matmuls and the kernel tail is
just the short o->h chain.
"""

import sys

sys.path.insert(0, "/opt/trn_rl_repo")

import numpy as np

import concourse.bacc as bacc
import concourse.mybir as mybir
import concourse.tile as tile
from concourse.bass_utils import run_bass_kernel_spmd

N_CORES = 8
B, D, U = 4096, 1024, 1024
BS = B // N_CORES          # batch rows per core
P = 128                    # SBUF partitions
KK = (D + U) // P          # 16 contraction blocks per Gauss stack
NJ = U // P                # 8 u-blocks
NSTACK = 3
PAIRS = ((1, 2), (0, 3))   # (f, c~), (i, o) by keras gate order i,f,c,o
KSUP = KK // 2             # weight DMA superblocks per pair-phase
WCOL = 2 * 2 * NSTACK * P  # 1536: [kk2, gl, stack, col]
GLAG = 4                   # k-step lag of gate gl=1 behind gl=0
F32 = mybir.dt.float32
F16 = mybir.dt.float16
ADD = mybir.AluOpType.add
MULT = mybir.AluOpType.mult
MIN = mybir.AluOpType.min

_CACHE = {}


def _build():
    nc = bacc.Bacc("TRN2", target_bir_lowering=False, debug=False,
                   num_devices=N_CORES)
    Tanh = mybir.ActivationFunctionType.Tanh
    Relu = mybir.ActivationFunctionType.Relu

    din = {}
    for name in ("a1T", "a2T", "a3T"):
        din[name] = nc.dram_tensor(name, [D + U, BS], F16,
                                   kind="ExternalInput").ap()
    din["c_prevT"] = nc.dram_tensor("c_prevT", [2 * U, BS], F32,
                                    kind="ExternalInput").ap()
    din["wq"] = nc.dram_tensor("wq", [NJ * 2 * KSUP * P, WCOL], F16,
                               kind="ExternalInput").ap()
    din["brT"] = nc.dram_tensor("brT", [P, 4 * NJ], F32,
                                kind="ExternalInput").ap()
    din["biT"] = nc.dram_tensor("biT", [P, 4 * NJ], F32,
                                kind="ExternalInput").ap()
    h_outT = nc.dram_tensor("h_outT", [2 * U, BS], F32, kind="ExternalOutput").ap()
    c_outT = nc.dram_tensor("c_outT", [2 * U, BS], F32, kind="ExternalOutput").ap()

    with tile.TileContext(nc) as tc:
        with (
            tc.tile_pool(name="acts", bufs=48) as acts,
            tc.tile_pool(name="bias", bufs=4) as bias_p,
            tc.tile_pool(name="wpool", bufs=14) as wpool,
            tc.tile_pool(name="cprev", bufs=16) as cpool,
            tc.tile_pool(name="gat", bufs=10) as gat_p,
            tc.tile_pool(name="tmp", bufs=12) as tmp_p,
            tc.tile_pool(name="outs", bufs=8) as out_p,
            tc.tile_pool(name="psum", bufs=8, space="PSUM") as psum_p,
        ):
            # --- resident fp16 moving blocks, one per (stack, k-block) ------
            act_tiles = {}

            def act(s, k):
                t = act_tiles.get((s, k))
                if t is None:
                    t = acts.tile([P, BS], F16, tag="acts", name=f"a{s}_{k}")
                    nc.sync.dma_start(t[:], din[f"a{s + 1}T"][k * P:(k + 1) * P, :])
                    act_tiles[(s, k)] = t
                return t

            # --- per-partition bias tiles [128, 32]; col = g*NJ + j ---------
            braw, bhs = [], []

            def emit_bias():
                for name in ("brT", "biT"):
                    t = bias_p.tile([P, 4 * NJ], F32, tag="bias",
                                    name=f"braw_{name}")
                    nc.sync.dma_start(t[:], din[name][:, :])
                    braw.append(t)
                    t2 = bias_p.tile([P, 4 * NJ], F32, tag="bias",
                                     name=f"bhs_{name}")
                    nc.vector.tensor_scalar(t2[:], t[:], 0.2, 0.5, MULT, ADD)
                    bhs.append(t2)

            cps = {}

            def cp(j, z):
                t = cps.get((j, z))
                if t is None:
                    rows0 = z * U + j * P
                    t = cpool.tile([P, BS], F32, tag="cprev", name=f"cp_{j}_{z}")
                    nc.sync.dma_start(t[:], din["c_prevT"][rows0:rows0 + P, :])
                    cps[(j, z)] = t
                return t

            # prime the first moving blocks so phase 0 doesn't outrun DMA
            for k in range(4):
                for s in range(NSTACK):
                    act(s, k)

            # per-j state carried from pair0 to pair1
            fgate = {}   # z -> min-able relu(f) tile
            tct = {}     # z -> tanh(c~) tile
            tc2s = {}    # z -> tanh(c_new) tile

            k1sb = {}

            def zpre(ps3, gl, z, j, g):
                # z-half pre-activation: k1 + (k3n if zr else k2). DVE can
                # only take one PSUM operand, so k1 goes through an ACT copy
                # to SBUF once per gate and is reused by both halves.
                k1 = k1sb.get((j, g))
                if k1 is None:
                    k1 = tmp_p.tile([P, BS], F32, tag="tmp", name=f"k1_{j}_{g}")
                    nc.scalar.copy(k1[:], ps3[(gl, 0)][:])
                    k1sb[(j, g)] = k1
                t = tmp_p.tile([P, BS], F32, tag="tmp", name=f"zp_{j}_{g}_{z}")
                other = 2 if z == 0 else 1
                nc.vector.tensor_tensor(t[:], k1[:], ps3[(gl, other)][:], ADD)
                return t

            def relugate(ps3, gl, z, j, g, chunk=None):
                # relu(0.2*z + (0.2*b + 0.5)); min(.,1) rides the consumer
                zp = zpre(ps3, gl, z, j, g)
                t = gat_p.tile([P, BS], F32, tag="gat", name=f"hs_{j}_{z}_{g}")
                bia = bhs[z][:, g * NJ + j:g * NJ + j + 1]
                nc.scalar.activation(t[:], zp[:], Relu, bias=bia, scale=0.2)
                return t

            for j in range(NJ):
                for pair in range(2):
                    gates = PAIRS[pair]
                    ps = {(gl, s): psum_p.tile([P, BS], F32, tag="ps",
                                               name=f"ps_{j}_{pair}_{gl}_{s}")
                          for gl in range(2) for s in range(NSTACK)}
                    wts = {}
                    for t in range(KK + GLAG):
                        if t < KK:
                            k = t
                            if k % 2 == 0:
                                ks = k // 2
                                wt = wpool.tile([P, WCOL], F16, tag="w",
                                                name=f"w_{j}_{pair}_{ks}")
                                row0 = ((j * 2 + pair) * KSUP + ks) * P
                                weng = (nc.scalar
                                        if (j == 0 and pair == 0 and ks < 2)
                                        else nc.sync)
                                weng.dma_start(wt[:], din["wq"][row0:row0 + P, :])
                                wts[ks] = wt
                            if j == 0 and pair == 0:
                                for s in range(NSTACK):
                                    act(s, k)
                            wt = wts[k // 2]
                            for s in range(NSTACK):
                                col0 = ((k % 2) * 2 * NSTACK + s) * P
                                nc.tensor.matmul(ps[(0, s)][:],
                                                 wt[:, col0:col0 + P],
                                                 act(s, k)[:],
                                                 start=(k == 0),
                                                 stop=(k == KK - 1))
                        if GLAG <= t:
                            k = t - GLAG
                            wt = wts[k // 2]
                            for s in range(NSTACK):
                                col0 = (((k % 2) * 2 + 1) * NSTACK + s) * P
                                nc.tensor.matmul(ps[(1, s)][:],
                                                 wt[:, col0:col0 + P],
                                                 act(s, k)[:],
                                                 start=(k == 0),
                                                 stop=(k == KK - 1))
                            if k % 2 == 1:
                                wts.pop(k // 2)
                        if pair == 0 and t == 2:
                            cp(j, 0)
                            cp(j, 1)
                        if t == KK - 1:
                            # gl=0 chains complete: f (pair0) or i (pair1)
                            if not braw:
                                emit_bias()
                            g = gates[0]
                            if pair == 0:
                                for z in range(2):
                                    fgate[z] = relugate(ps, 0, z, j, g)
                            else:
                                for z in range(2):
                                    i_t = relugate(ps, 0, z, j, g)
                                    # c = min(f,1)*c_prev + min(i,1)*tanh(c~)
                                    t1 = tmp_p.tile([P, BS], F32, tag="tmp",
                                                    name=f"t1_{j}_{z}")
                                    nc.vector.scalar_tensor_tensor(
                                        t1[:], fgate[z][:], 1.0, cp(j, z)[:],
                                        MIN, MULT)
                                    t2 = tmp_p.tile([P, BS], F32, tag="tmp",
                                                    name=f"t2_{j}_{z}")
                                    nc.vector.scalar_tensor_tensor(
                                        t2[:], i_t[:], 1.0, tct[z][:],
                                        MIN, MULT)
                                    cn = out_p.tile([P, BS], F32, tag="out",
                                                    name=f"cn_{j}_{z}")
                                    nc.vector.tensor_tensor(cn[:], t1[:],
                                                            t2[:], ADD)
                                    rows0 = z * U + j * P
                                    nc.sync.dma_start(
                                        c_outT[rows0:rows0 + P, :], cn[:])
                                    tc2 = tmp_p.tile([P, BS], F32, tag="tmp",
                                                     name=f"tc2_{j}_{z}")
                                    nc.scalar.activation(tc2[:], cn[:], Tanh)
                                    tc2s[z] = tc2
                    # gl=1 chains complete at loop end: c~ (pair0), o (pair1)
                    g = gates[1]
                    if pair == 0:
                        for z in range(2):
                            zp = zpre(ps, 1, z, j, g)
                            tt = tmp_p.tile([P, BS], F32, tag="tmp",
                                            name=f"tct_{j}_{z}")
                            bia = braw[z][:, g * NJ + j:g * NJ + j + 1]
                            nc.scalar.activation(tt[:], zp[:], Tanh,
                                                 bias=bia, scale=1.0)
                            tct[z] = tt
                    else:
                        for z in range(2):
                            rows0 = z * U + j * P
                            if j == NJ - 1 and z == 1:
                                # kernel tail: half-batch chunks pipeline the
                                # ACT relu, DVE mul and h DMA
                                zp = zpre(ps, 1, z, j, g)
                                o_t = gat_p.tile([P, BS], F32, tag="gat",
                                                 name=f"hsl_{j}_{z}")
                                hn = out_p.tile([P, BS], F32, tag="out",
                                                name=f"hn_{j}_{z}")
                                bia = bhs[z][:, g * NJ + j:g * NJ + j + 1]
                                for h0 in (0, BS // 2):
                                    sl = slice(h0, h0 + BS // 2)
                                    nc.scalar.activation(
                                        o_t[:, sl], zp[:, sl], Relu,
                                        bias=bia, scale=0.2)
                                    nc.vector.scalar_tensor_tensor(
                                        hn[:, sl], o_t[:, sl], 1.0,
                                        tc2s[z][:, sl], MIN, MULT)
                                    nc.sync.dma_start(
                                        h_outT[rows0:rows0 + P, sl],
                                        hn[:, sl])
                            else:
                                o_t = relugate(ps, 1, z, j, g)
                                hn = out_p.tile([P, BS], F32, tag="out",
                                                name=f"hn_{j}_{z}")
                                nc.vector.scalar_tensor_tensor(
                                    hn[:], o_t[:], 1.0, tc2s[z][:],
                                    MIN, MULT)
                                nc.sync.dma_start(
                                    h_outT[rows0:rows0 + P, :], hn[:])

    nc.compile()
    return nc


def _in_maps(inputs, h_tm1, c_tm1, wr, wi, wrr, wir, br, bi):
    brT = np.ascontiguousarray(br.reshape(4 * NJ, P).T)
    biT = np.ascontiguousarray(bi.reshape(4 * NJ, P).T)
    # Gauss weight stacks, fp16: k1 | k2 | k3n
    W1 = np.concatenate([wr, wrr], 0)
    W2 = np.concatenate([-(wi + wr), -(wir + wrr)], 0)
    W3 = np.concatenate([wi - wr, wir - wrr], 0)
    Ws = np.stack([W1, W2, W3]).astype(np.float16)       # [s, 2048, 4096]
    v = Ws.reshape(NSTACK, KK, P, 4, NJ, P)              # [s, kk, p, g, j, c]
    vp = v[:, :, :, (1, 2, 0, 3), :, :]                  # gate order by pair
    vp = vp.reshape(NSTACK, KSUP, 2, P, 2, 2, NJ, P)     # [s,ks,kk2,p,pair,gl,j,c]
    wq = np.ascontiguousarray(
        vp.transpose(6, 4, 1, 3, 2, 5, 0, 7).reshape(NJ * 2 * KSUP * P, WCOL))

    maps = []
    for c in range(N_CORES):
        rows = slice(c * BS, (c + 1) * BS)
        xr, xi_ = inputs[rows, :D], inputs[rows, D:]
        hr, hi = h_tm1[rows, :U], h_tm1[rows, U:]
        a1 = np.empty((D + U, BS), np.float16)
        a1[:D] = (xr + xi_).T
        a1[D:] = (hr + hi).T
        a2 = np.empty((D + U, BS), np.float16)
        a2[:D] = xr.T
        a2[D:] = hr.T
        a3 = np.empty((D + U, BS), np.float16)
        a3[:D] = xi_.T
        a3[D:] = hi.T
        maps.append({
            "a1T": a1, "a2T": a2, "a3T": a3,
            "c_prevT": np.ascontiguousarray(c_tm1[rows].T),
            "wq": wq,
            "brT": brT, "biT": biT,
        })
    return maps


def kernel(inputs, h_tm1, c_tm1, real_kernel, imaginary_kernel,
           real_recurrent_kernel, imaginary_recurrent_kernel,
           real_bias, imaginary_bias):
    if "nc" not in _CACHE:
        _CACHE["nc"] = _build()
    nc = _CACHE["nc"]

    maps = _in_maps(
        np.ascontiguousarray(inputs, dtype=np.float32),
        np.ascontiguousarray(h_tm1, dtype=np.float32),
        np.ascontiguousarray(c_tm1, dtype=np.float32),
        np.ascontiguousarray(real_kernel, dtype=np.float32),
        np.ascontiguousarray(imaginary_kernel, dtype=np.float32),
        np.ascontiguousarray(real_recurrent_kernel, dtype=np.float32),
        np.ascontiguousarray(imaginary_recurrent_kernel, dtype=np.float32),
        np.ascontiguousarray(real_bias, dtype=np.float32),
        np.ascontiguousarray(imaginary_bias, dtype=np.float32),
    )
    res = run_bass_kernel_spmd(nc, maps, list(range(N_CORES)))
    h = np.concatenate(
        [res.results[c]["h_outT"].T for c in range(N_CORES)], axis=0)
    c = np.concatenate(
        [res.results[c]["c_outT"].T for c in range(N_CORES)], axis=0)
    return np.ascontiguousarray(h), np.ascontiguousarray(c)


# revision 9
# speedup vs baseline: 1.3205x; 1.0113x over previous
"""CLSTMCell fused cell kernel for 8 Trainium2 NeuronCores.

Data-parallel over the batch: each of the 8 cores processes a 512-row batch
shard; weights and biases are replicated to every core.

Complex-multiply structure (z = x·(R - iI) + h·(Rr - iIr) + b) is computed
with Gauss's 3-multiplication trick instead of 4 real matmuls:
    k1  = (xr+xi)@R + (hr+hi)@Rr
    k2  = xr@(-(I+R)) + hr@(-(Ir+Rr))
    k3n = xi@(I-R)    + hi@(Ir-Rr)
    zr = k1 + k3n + br          [512, 4096]
    zi = k1 + k2  + bi          [512, 4096]
This cuts tensor-engine work by 25%. All matmul operands are fp16 (e5m10),
which halves weight DMA vs fp32 while keeping the end-to-end max-rel error
at ~2e-3 (measured on the reference data), well inside the 2e-2 gate. The
weight combinations and the (x+xi)/(h+hi) sums are precomputed on host.

Device layout: output columns (n) on SBUF partitions, batch on the free dim.
One phase = one (128-wide u-block, gate-pair): pair0=(f,c~), pair1=(i,o).
Per gate, the three Gauss accumulation chains live in 3 PSUM banks (6 banks
per pair-phase); the chains contract over 16 k-blocks of 128. The gate
combine reads two PSUM banks per z-half with one DVE add, then applies the
activation on ACT with fused scale+bias. Gate gl=1 trails gl=0 by GLAG
k-steps so combines overlap the next chains' matmuls and the kernel tail is
just the short o->h chain.
"""

import sys

sys.path.insert(0, "/opt/trn_rl_repo")

import numpy as np

import concourse.bacc as bacc
import concourse.mybir as mybir
import concourse.tile as tile
from concourse.bass_utils import run_bass_kernel_spmd

N_CORES = 8
B, D, U = 4096, 1024, 1024
BS = B // N_CORES          # batch rows per core
P = 128                    # SBUF partitions
KK = (D + U) // P          # 16 contraction blocks per Gauss stack
NJ = U // P                # 8 u-blocks
NSTACK = 3
PAIRS = ((1, 2), (0, 3))   # (f, c~), (i, o) by keras gate order i,f,c,o
KSUP = KK // 2             # weight DMA superblocks per pair-phase
WCOL = 2 * 2 * NSTACK * P  # 1536: [kk2, gl, stack, col]
GLAG = 4                   # k-step lag of gate gl=1 behind gl=0
F32 = mybir.dt.float32
F16 = mybir.dt.float16
ADD = mybir.AluOpType.add
MULT = mybir.AluOpType.mult
MIN = mybir.AluOpType.min

_CACHE = {}


def _build():
    nc = bacc.Bacc("TRN2", target_bir_lowering=False, debug=False,
                   num_devices=N_CORES)
    Tanh = mybir.ActivationFunctionType.Tanh
    Relu = mybir.ActivationFunctionType.Relu

    din = {}
    for name in ("a1T", "a2T", "a3T"):
        din[name] = nc.dram_tensor(name, [D + U, BS], F16,
                                   kind="ExternalInput").ap()
    din["c_prevT"] = nc.dram_tensor("c_prevT", [2 * U, BS], F32,
                                    kind="ExternalInput").ap()
    din["wq"] = nc.dram_tensor("wq", [NJ * 2 * KSUP * P, WCOL], F16,
                               kind="ExternalInput").ap()
    din["brT"] = nc.dram_tensor("brT", [P, 4 * NJ], F32,
                                kind="ExternalInput").ap()
    din["biT"] = nc.dram_tensor("biT", [P, 4 * NJ], F32,
                                kind="ExternalInput").ap()
    h_outT = nc.dram_tensor("h_outT", [2 * U, BS], F32, kind="ExternalOutput").ap()
    c_outT = nc.dram_tensor("c_outT", [2 * U, BS], F32, kind="ExternalOutput").ap()

    with tile.TileContext(nc) as tc:
        with (
            tc.tile_pool(name="acts", bufs=48) as acts,
            tc.tile_pool(name="bias", bufs=4) as bias_p,
            tc.tile_pool(name="wpool", bufs=14) as wpool,
            tc.tile_pool(name="cprev", bufs=16) as cpool,
            tc.tile_pool(name="gat", bufs=10) as gat_p,
            tc.tile_pool(name="tmp", bufs=12) as tmp_p,
            tc.tile_pool(name="outs", bufs=8) as out_p,
            tc.tile_pool(name="psum", bufs=8, space="PSUM") as psum_p,
        ):
            # --- resident fp16 moving blocks, one per (stack, k-block) ------
            act_tiles = {}

            def act(s, k):
                t = act_tiles.get((s, k))
                if t is None:
                    t = acts.tile([P, BS], F16, tag="acts", name=f"a{s}_{k}")
                    nc.sync.dma_start(t[:], din[f"a{s + 1}T"][k * P:(k + 1) * P, :])
                    act_tiles[(s, k)] = t
                return t

            # --- per-partition bias tiles [128, 32]; col = g*NJ + j ---------
            braw, bhs = [], []

            def emit_bias():
                for name in ("brT", "biT"):
                    t = bias_p.tile([P, 4 * NJ], F32, tag="bias",
                                    name=f"braw_{name}")
                    nc.sync.dma_start(t[:], din[name][:, :])
                    braw.append(t)
                    t2 = bias_p.tile([P, 4 * NJ], F32, tag="bias",
                                     name=f"bhs_{name}")
                    nc.vector.tensor_scalar(t2[:], t[:], 0.2, 0.5, MULT, ADD)
                    bhs.append(t2)

            cps = {}

            def cp(j, z):
                t = cps.get((j, z))
                if t is None:
                    rows0 = z * U + j * P
                    t = cpool.tile([P, BS], F32, tag="cprev", name=f"cp_{j}_{z}")
                    nc.sync.dma_start(t[:], din["c_prevT"][rows0:rows0 + P, :])
                    cps[(j, z)] = t
                return t

            # biases first: tiny DMAs that must not queue behind the weight
            # stream — the first gate combines (and so PSUM-bank recycling)
            # depend on them
            emit_bias()

            # prime the first moving blocks so phase 0 doesn't outrun DMA
            for k in range(3):
                for s in range(NSTACK):
                    act(s, k)

            # per-j state carried from pair0 to pair1
            fgate = {}   # z -> min-able relu(f) tile
            tct = {}     # z -> tanh(c~) tile
            tc2s = {}    # z -> tanh(c_new) tile

            k1sb = {}

            def k1copy(ps3, gl, j, g):
                # DVE can only take one PSUM operand, so k1 goes through an
                # ACT copy to SBUF once per gate, reused by both halves
                k1 = k1sb.get((j, g))
                if k1 is None:
                    k1 = tmp_p.tile([P, BS], F32, tag="tmp", name=f"k1_{j}_{g}")
                    nc.scalar.copy(k1[:], ps3[(gl, 0)][:])
                    k1sb[(j, g)] = k1
                return k1

            def zpre(ps3, gl, z, j, g):
                # z-half pre-activation: k1 + (k3n if zr else k2)
                k1 = k1copy(ps3, gl, j, g)
                t = tmp_p.tile([P, BS], F32, tag="tmp", name=f"zp_{j}_{g}_{z}")
                other = 2 if z == 0 else 1
                nc.vector.tensor_tensor(t[:], k1[:], ps3[(gl, other)][:], ADD)
                return t

            def relugate(ps3, gl, z, j, g, chunk=None):
                # relu(0.2*z + (0.2*b + 0.5)); min(.,1) rides the consumer
                zp = zpre(ps3, gl, z, j, g)
                t = gat_p.tile([P, BS], F32, tag="gat", name=f"hs_{j}_{z}_{g}")
                bia = bhs[z][:, g * NJ + j:g * NJ + j + 1]
                nc.scalar.activation(t[:], zp[:], Relu, bias=bia, scale=0.2)
                return t

            for j in range(NJ):
                for pair in range(2):
                    gates = PAIRS[pair]
                    last = j == NJ - 1 and pair == 1
                    # bigger lag in the last phase buys the i->c chain time
                    # to finish under the o-gate matmuls; the o-gate's k1
                    # stack additionally leads by 2 slots so its PSUM->SBUF
                    # copy hides under the k2/k3n tail matmuls
                    glag = 2 * GLAG if last else GLAG
                    slags = (glag - 2, glag, glag) if last else (glag,) * 3
                    ps = {(gl, s): psum_p.tile([P, BS], F32, tag="ps",
                                               name=f"ps_{j}_{pair}_{gl}_{s}")
                          for gl in range(2) for s in range(NSTACK)}
                    wts = {}
                    for t in range(KK + glag):
                        if t < KK:
                            k = t
                            if k % 2 == 0:
                                ks = k // 2
                                wt = wpool.tile([P, WCOL], F16, tag="w",
                                                name=f"w_{j}_{pair}_{ks}")
                                row0 = ((j * 2 + pair) * KSUP + ks) * P
                                weng = (nc.scalar
                                        if (j == 0 and pair == 0 and ks < 2)
                                        else nc.sync)
                                weng.dma_start(wt[:], din["wq"][row0:row0 + P, :])
                                wts[ks] = wt
                            if j == 0 and pair == 0:
                                for s in range(NSTACK):
                                    act(s, k)
                            wt = wts[k // 2]
                            for s in range(NSTACK):
                                col0 = ((k % 2) * 2 * NSTACK + s) * P
                                nc.tensor.matmul(ps[(0, s)][:],
                                                 wt[:, col0:col0 + P],
                                                 act(s, k)[:],
                                                 start=(k == 0),
                                                 stop=(k == KK - 1))
                        for s in range(NSTACK):
                            if slags[s] <= t < KK + slags[s]:
                                k = t - slags[s]
                                wt = wts[k // 2]
                                col0 = (((k % 2) * 2 + 1) * NSTACK + s) * P
                                nc.tensor.matmul(ps[(1, s)][:],
                                                 wt[:, col0:col0 + P],
                                                 act(s, k)[:],
                                                 start=(k == 0),
                                                 stop=(k == KK - 1))
                        km = t - glag
                        if 0 <= km < KK and km % 2 == 1:
                            wts.pop(km // 2)
                        if last and t == KK - 1 + slags[0]:
                            # o-gate k1 copy hides under the k2/k3n tail mms
                            k1copy(ps, 1, j, gates[1])
                        if pair == 0 and t == 2:
                            cp(j, 0)
                            cp(j, 1)
                        if t == KK - 1:
                            # gl=0 chains complete: f (pair0) or i (pair1)
                            if not braw:
                                emit_bias()
                            g = gates[0]
                            if pair == 0:
                                for z in range(2):
                                    fgate[z] = relugate(ps, 0, z, j, g)
                            else:
                                for z in range(2):
                                    i_t = relugate(ps, 0, z, j, g)
                                    # c = min(f,1)*c_prev + min(i,1)*tanh(c~)
                                    t1 = tmp_p.tile([P, BS], F32, tag="tmp",
                                                    name=f"t1_{j}_{z}")
                                    nc.vector.scalar_tensor_tensor(
                                        t1[:], fgate[z][:], 1.0, cp(j, z)[:],
                                        MIN, MULT)
                                    t2 = tmp_p.tile([P, BS], F32, tag="tmp",
                                                    name=f"t2_{j}_{z}")
                                    nc.vector.scalar_tensor_tensor(
                                        t2[:], i_t[:], 1.0, tct[z][:],
                                        MIN, MULT)
                                    cn = out_p.tile([P, BS], F32, tag="out",
                                                    name=f"cn_{j}_{z}")
                                    nc.vector.tensor_tensor(cn[:], t1[:],
                                                            t2[:], ADD)
                                    rows0 = z * U + j * P
                                    nc.sync.dma_start(
                                        c_outT[rows0:rows0 + P, :], cn[:])
                                    tc2 = tmp_p.tile([P, BS], F32, tag="tmp",
                                                     name=f"tc2_{j}_{z}")
                                    nc.scalar.activation(tc2[:], cn[:], Tanh)
                                    tc2s[z] = tc2
                    # gl=1 chains complete at loop end: c~ (pair0), o (pair1)
                    g = gates[1]
                    if pair == 0:
                        for z in range(2):
                            zp = zpre(ps, 1, z, j, g)
                            tt = tmp_p.tile([P, BS], F32, tag="tmp",
                                            name=f"tct_{j}_{z}")
                            bia = braw[z][:, g * NJ + j:g * NJ + j + 1]
                            nc.scalar.activation(tt[:], zp[:], Tanh,
                                                 bias=bia, scale=1.0)
                            tct[z] = tt
                    elif last:
                        # kernel tail: half-batch chunks pipeline the DVE
                        # add, ACT relu, DVE mul and h DMA for both halves
                        k1 = k1copy(ps, 1, j, g)
                        for z in range(2):
                            rows0 = z * U + j * P
                            other = 2 if z == 0 else 1
                            bia = bhs[z][:, g * NJ + j:g * NJ + j + 1]
                            zp = tmp_p.tile([P, BS], F32, tag="tmp",
                                            name=f"zpl_{j}_{z}")
                            o_t = gat_p.tile([P, BS], F32, tag="gat",
                                             name=f"hsl_{j}_{z}")
                            hn = out_p.tile([P, BS], F32, tag="out",
                                            name=f"hn_{j}_{z}")
                            for h0 in (0, BS // 2):
                                sl = slice(h0, h0 + BS // 2)
                                nc.vector.tensor_tensor(
                                    zp[:, sl], k1[:, sl],
                                    ps[(1, other)][:, sl], ADD)
                                nc.scalar.activation(
                                    o_t[:, sl], zp[:, sl], Relu,
                                    bias=bia, scale=0.2)
                                nc.vector.scalar_tensor_tensor(
                                    hn[:, sl], o_t[:, sl], 1.0,
                                    tc2s[z][:, sl], MIN, MULT)
                                nc.sync.dma_start(
                                    h_outT[rows0:rows0 + P, sl], hn[:, sl])
                    else:
                        for z in range(2):
                            rows0 = z * U + j * P
                            o_t = relugate(ps, 1, z, j, g)
                            hn = out_p.tile([P, BS], F32, tag="out",
                                            name=f"hn_{j}_{z}")
                            nc.vector.scalar_tensor_tensor(
                                hn[:], o_t[:], 1.0, tc2s[z][:],
                                MIN, MULT)
                            nc.sync.dma_start(
                                h_outT[rows0:rows0 + P, :], hn[:])

    nc.compile()
    return nc


def _in_maps(inputs, h_tm1, c_tm1, wr, wi, wrr, wir, br, bi):
    brT = np.ascontiguousarray(br.reshape(4 * NJ, P).T)
    biT = np.ascontiguousarray(bi.reshape(4 * NJ, P).T)
    # Gauss weight stacks, fp16: k1 | k2 | k3n
    W1 = np.concatenate([wr, wrr], 0)
    W2 = np.concatenate([-(wi + wr), -(wir + wrr)], 0)
    W3 = np.concatenate([wi - wr, wir - wrr], 0)
    Ws = np.stack([W1, W2, W3]).astype(np.float16)       # [s, 2048, 4096]
    v = Ws.reshape(NSTACK, KK, P, 4, NJ, P)              # [s, kk, p, g, j, c]
    vp = v[:, :, :, (1, 2, 0, 3), :, :]                  # gate order by pair
    vp = vp.reshape(NSTACK, KSUP, 2, P, 2, 2, NJ, P)     # [s,ks,kk2,p,pair,gl,j,c]
    wq = np.ascontiguousarray(
        vp.transpose(6, 4, 1, 3, 2, 5, 0, 7).reshape(NJ * 2 * KSUP * P, WCOL))

    maps = []
    for c in range(N_CORES):
        rows = slice(c * BS, (c + 1) * BS)
        xr, xi_ = inputs[rows, :D], inputs[rows, D:]
        hr, hi = h_tm1[rows, :U], h_tm1[rows, U:]
        a1 = np.empty((D + U, BS), np.float16)
        a1[:D] = (xr + xi_).T
        a1[D:] = (hr + hi).T
        a2 = np.empty((D + U, BS), np.float16)
        a2[:D] = xr.T
        a2[D:] = hr.T
        a3 = np.empty((D + U, BS), np.float16)
        a3[:D] = xi_.T
        a3[D:] = hi.T
        maps.append({
            "a1T": a1, "a2T": a2, "a3T": a3,
            "c_prevT": np.ascontiguousarray(c_tm1[rows].T),
            "wq": wq,
            "brT": brT, "biT": biT,
        })
    return maps


def kernel(inputs, h_tm1, c_tm1, real_kernel, imaginary_kernel,
           real_recurrent_kernel, imaginary_recurrent_kernel,
           real_bias, imaginary_bias):
    if "nc" not in _CACHE:
        _CACHE["nc"] = _build()
    nc = _CACHE["nc"]

    maps = _in_maps(
        np.ascontiguousarray(inputs, dtype=np.float32),
        np.ascontiguousarray(h_tm1, dtype=np.float32),
        np.ascontiguousarray(c_tm1, dtype=np.float32),
        np.ascontiguousarray(real_kernel, dtype=np.float32),
        np.ascontiguousarray(imaginary_kernel, dtype=np.float32),
        np.ascontiguousarray(real_recurrent_kernel, dtype=np.float32),
        np.ascontiguousarray(imaginary_recurrent_kernel, dtype=np.float32),
        np.ascontiguousarray(real_bias, dtype=np.float32),
        np.ascontiguousarray(imaginary_bias, dtype=np.float32),
    )
    res = run_bass_kernel_spmd(nc, maps, list(range(N_CORES)))
    h = np.concatenate(
        [res.results[c]["h_outT"].T for c in range(N_CORES)], axis=0)
    c = np.concatenate(
        [res.results[c]["c_outT"].T for c in range(N_CORES)], axis=0)
    return np.ascontiguousarray(h), np.ascontiguousarray(c)
